# revision 2
# baseline (speedup 1.0000x reference)
"""Trainium2 kernel for nn_ConvTrace: batch of 64 graphs, conv -> traces of
matrix powers -> coef-weighted sum.

Pipeline:
- Host: 6x6 conv via im2col GEMM (BLAS), zero-pad 251->256, round to bf16
  (RNE), pack natural+transposed layouts into one interleaved buffer, and
  compute t2 = tr(C^2) = <C, C^T> in full precision.
- Device (8 NeuronCores, data-parallel over the batch, 64 (b,ch) pairs/core):
  per pair, two bf16 matmul products on the PE (N=256 -> 1 cyc/row):
  D = C2^T = mm(lhsT=Cn, rhs=Ct) and C3 = C2@C = mm(lhsT=ds, rhs=Cn).
  ScalarE copies both PSUM results to SBUF bf16 (ds, c3). All three traces
  as DVE fused multiply-accumulate dots in bf16 2x mode:
  t3 = <ds, Cn>, t4 = <c3, Ct>, t5 = <c3, ds>. Per-partition partials DMA'd
  out once at the end.
- Host: reduce partials over partitions in float64 and apply the power/coef
  math.
"""

import os
from contextlib import ExitStack

import numpy as np

B = 64
G = 256
KK = 6
CH = 8
ROWS = 4
COLS = 3
H = G - KK + 1  # 251
NCORES = 8
PAIRS_PER_CORE = (B // NCORES) * CH  # 64

_COMPILED = None
LAST_EXEC_NS = None


def _build():
    """Build + compile the SPMD bass kernel once per process."""
    global _COMPILED
    if _COMPILED is not None:
        return _COMPILED

    import concourse.bacc as bacc
    import concourse.tile as tile
    from concourse import mybir

    F32 = mybir.dt.float32
    BF16 = mybir.dt.bfloat16
    npair = PAIRS_PER_CORE

    nc = bacc.Bacc(None, target_bir_lowering=False)
    # interleaved natural (which=0) + transposed (which=1) layouts, one DMA/pair
    cc_d = nc.declare_dram_parameter("cc", [npair, 128, 2, 2, 256], BF16, isOutput=False)
    pa_d = nc.declare_dram_parameter("pa", [128, npair * 3], F32, isOutput=True)

    with tile.TileContext(nc) as tc, ExitStack() as ctx:
        inp = ctx.enter_context(tc.tile_pool(name="inp", bufs=8))
        sb_ds = ctx.enter_context(tc.tile_pool(name="sb_ds", bufs=4))
        sb_c3 = ctx.enter_context(tc.tile_pool(name="sb_c3", bufs=4))
        scr = ctx.enter_context(tc.tile_pool(name="scr", bufs=6))
        pp = ctx.enter_context(tc.tile_pool(name="pp", bufs=1))
        ps_d = ctx.enter_context(tc.tile_pool(name="ps_d", bufs=3, space="PSUM"))
        ps_c3 = ctx.enter_context(tc.tile_pool(name="ps_c3", bufs=3, space="PSUM"))

        partials = pp.tile([128, npair * 3], F32)

        for pair in range(npair):
            cct = inp.tile([128, 2, 2, 256], BF16, tag="cc")
            nc.sync.dma_start(out=cct[:], in_=cc_d[pair])
            cn = cct[:, 0]
            ct = cct[:, 1]

            def mm4(out_ps, lhs_t, rhs_t):
                # one PSUM accumulation group spanning the whole bank
                for i, (q, kt) in enumerate(((0, 0), (1, 0), (0, 1), (1, 1))):
                    nc.tensor.matmul(
                        out_ps[:, q, :],
                        lhs_t[:, kt, q * 128:(q + 1) * 128],
                        rhs_t[:, kt, :],
                        start=(i == 0),
                        stop=(i == 3),
                    )

            # D = C2^T = mm(cn, ct); C3 = C2@C = mm(ds, cn)
            pd = ps_d.tile([128, 2, 256], F32)
            mm4(pd, cn, ct)
            ds = sb_ds.tile([128, 2, 256], BF16, tag="ds")
            nc.scalar.copy(ds[:], pd[:])

            pc3 = ps_c3.tile([128, 2, 256], F32)
            mm4(pc3, ds, cn)
            c3 = sb_c3.tile([128, 2, 256], BF16, tag="c3")
            nc.scalar.copy(c3[:], pc3[:])

            def dot(col, a, b):
                out = scr.tile([128, 2, 256], BF16, tag="scr")
                nc.vector.scalar_tensor_tensor(
                    out=out[:],
                    in0=a,
                    scalar=1.0,
                    in1=b,
                    op0=mybir.AluOpType.mult,
                    op1=mybir.AluOpType.mult,
                    accum_out=partials[:, col:col + 1],
                )

            dot(pair * 3 + 0, ds[:], cn)       # t3 = <C2^T, C>
            dot(pair * 3 + 1, c3[:], ct)       # t4 = <C3, C^T>
            dot(pair * 3 + 2, c3[:], ds[:])    # t5 = <C3, C2^T>

        nc.sync.dma_start(out=pa_d[:], in_=partials[:])

    nc.compile()
    _COMPILED = nc
    return nc


def kernel(x, conv_w, conv_b, coef):
    global LAST_EXEC_NS
    import ml_dtypes

    x = np.asarray(x, dtype=np.float32)
    conv_w = np.asarray(conv_w, dtype=np.float32)
    conv_b = np.asarray(conv_b, dtype=np.float32)
    coef = np.asarray(coef, dtype=np.float32)

    # --- host: conv via im2col GEMM ---
    from numpy.lib.stride_tricks import sliding_window_view
    win = sliding_window_view(x, (KK, KK), axis=(1, 2))      # [B,H,H,KK,KK]
    patches = np.ascontiguousarray(win).reshape(B, H * H, KK * KK)
    wmat = conv_w.reshape(CH, KK * KK)
    C = patches @ wmat.T                                      # [B, H*H, CH]
    C = C.transpose(0, 2, 1).reshape(B, CH, H, H) + conv_b[None, :, None, None]

    Cpad = np.zeros((B * CH, 256, 256), np.float32)
    Cpad[:, :H, :H] = C.reshape(B * CH, H, H)

    # t2 in full precision on host (the dominant-cancellation trace)
    t2 = np.einsum("pij,pji->p", Cpad.astype(np.float64), Cpad.astype(np.float64))

    # round once to bf16 (RNE), build both layouts from the same rounded values
    Cb = Cpad.astype(ml_dtypes.bfloat16)                      # [512,256,256]
    n = B * CH
    cn = Cb.reshape(n, 2, 128, 256).transpose(0, 2, 1, 3)
    ct = np.ascontiguousarray(Cb.transpose(0, 2, 1)).reshape(
        n, 2, 128, 256).transpose(0, 2, 1, 3)
    cc = np.ascontiguousarray(
        np.stack([cn, ct], axis=2))                           # [n,128,2,2,256]

    nc = _build()
    from concourse.bass_utils import run_bass_kernel_spmd

    npair = PAIRS_PER_CORE
    in_maps = [{"cc": cc[c * npair:(c + 1) * npair]} for c in range(NCORES)]

    trace = os.environ.get("CONVTRACE_PROFILE", "0") == "1"
    if trace:
        import sys
        import types
        if "antenv.axon_hooks" not in sys.modules:
            import antenv  # noqa: F401
            from trn_agent_boot.trn_boot import _ntff_profile_via_ctypes
            hook = _ntff_profile_via_ctypes("/opt/axon/libaxon_pjrt.so")
            mod = types.ModuleType("antenv.axon_hooks")
            mod.get_axon_ntff_profile_hook = lambda: hook
            mod.set_axon_ntff_profile_hook = lambda h: None
            sys.modules["antenv.axon_hooks"] = mod
        import concourse.bass_utils as bu
        bu.upload_artifacts = lambda tmpdir: tmpdir

    res = run_bass_kernel_spmd(nc, in_maps, list(range(NCORES)), trace=trace)
    LAST_EXEC_NS = res.exec_time_ns

    # --- host: finalize in float64 ---
    ts = np.empty((B * CH, 4), np.float64)
    ts[:, 0] = t2
    for c in range(NCORES):
        pa = res.results[c]["pa"].astype(np.float64)           # [128, npair*3]
        t345 = pa.sum(axis=0).reshape(npair, 3)
        ts[c * npair:(c + 1) * npair, 1:] = t345

    ts = ts.reshape(B, CH, 4)
    jpow = np.arange(1, COLS + 1, dtype=np.float64)
    retm = ts[..., None] ** jpow                               # [B,CH,ROWS,COLS]
    exps = (np.arange(ROWS, dtype=np.float64)[:, None]
            + np.arange(COLS, dtype=np.float64)[None, :] + 1.0)
    retm = retm / (np.float64(H * H) ** exps)
    out = (coef.astype(np.float64)[None] * retm).sum(axis=(1, 2, 3))
    return out.astype(np.float32)


# revision 7
# speedup vs baseline: 1.1053x; 1.1053x over previous
"""Trainium2 kernel for nn_ConvTrace: batch of 64 graphs, conv -> traces of
matrix powers -> coef-weighted sum.

Pipeline:
- Host: 6x6 conv via im2col GEMM (BLAS), zero-pad 251->256, round to bf16
  (RNE), pack natural+transposed layouts into one interleaved buffer, and
  compute t2 = tr(C^2) = <C, C^T> in full precision.
- Device (8 NeuronCores, data-parallel over the batch, 64 (b,ch) pairs/core):
  per pair, two bf16 matmul products on the PE (N=256 -> 1 cyc/row):
  D = C2^T = mm(lhsT=Cn, rhs=Ct) and C3 = C2@C = mm(lhsT=ds, rhs=Cn).
  ScalarE copies both PSUM results to SBUF bf16 (ds, c3). All three traces
  as DVE fused multiply-accumulate dots in bf16 2x mode:
  t3 = <ds, Cn>, t4 = <c3, Ct>, t5 = <c3, ds>. Per-partition partials DMA'd
  out once at the end.
- Host: reduce partials over partitions in float64 and apply the power/coef
  math.
"""

import os
from contextlib import ExitStack

import numpy as np

B = 64
G = 256
KK = 6
CH = 8
ROWS = 4
COLS = 3
H = G - KK + 1  # 251
NCORES = 8
PAIRS_PER_CORE = (B // NCORES) * CH  # 64

_COMPILED = None
LAST_EXEC_NS = None


def _build():
    """Build + compile the SPMD bass kernel once per process."""
    global _COMPILED
    if _COMPILED is not None:
        return _COMPILED

    import concourse.bacc as bacc
    import concourse.tile as tile
    from concourse import mybir

    F32 = mybir.dt.float32
    BF16 = mybir.dt.bfloat16
    npair = PAIRS_PER_CORE

    UI16 = mybir.dt.uint16

    nc = bacc.Bacc(None, target_bir_lowering=False)
    # interleaved natural (which=0) + transposed (which=1) layouts, one DMA/pair
    cc_d = nc.declare_dram_parameter("cc", [npair, 128, 2, 2, 256], BF16, isOutput=False)
    oh_d = nc.declare_dram_parameter("oh", [128, 127], BF16, isOutput=False)
    pa_d = nc.declare_dram_parameter("pa", [128, npair * 2], F32, isOutput=True)
    dg_d = nc.declare_dram_parameter("dg", [64, 512], F32, isOutput=True)

    with tile.TileContext(nc) as tc, ExitStack() as ctx:
        inp = ctx.enter_context(tc.tile_pool(name="inp", bufs=10))
        sb_ds = ctx.enter_context(tc.tile_pool(name="sb_ds", bufs=4))
        sb_c3 = ctx.enter_context(tc.tile_pool(name="sb_c3", bufs=4))
        scr = ctx.enter_context(tc.tile_pool(name="scr", bufs=6))
        pp = ctx.enter_context(tc.tile_pool(name="pp", bufs=1))
        ps_d = ctx.enter_context(tc.tile_pool(name="ps_d", bufs=3, space="PSUM"))
        ps_c3 = ctx.enter_context(tc.tile_pool(name="ps_c3", bufs=3, space="PSUM"))

        ps_t3 = ctx.enter_context(tc.tile_pool(name="ps_t3", bufs=1, space="PSUM"))
        partials = pp.tile([128, npair * 2], F32)
        t3sb = pp.tile([64, 512], F32)
        oht = pp.tile([128, 127], BF16)
        nc.sync.dma_start(out=oht[:], in_=oh_d[:])
        pt3 = ps_t3.tile([64, 512], F32)

        for pair in range(npair):
            cct = inp.tile([128, 2, 2, 256], BF16, tag="cc")
            nc.sync.dma_start(out=cct[:], in_=cc_d[pair])
            cn = cct[:, 0]
            ct = cct[:, 1]

            def mm4(out_ps, lhs_t, rhs_t):
                # one PSUM accumulation group spanning the whole bank
                for i, (q, kt) in enumerate(((0, 0), (1, 0), (0, 1), (1, 1))):
                    nc.tensor.matmul(
                        out_ps[:, q, :],
                        lhs_t[:, kt, q * 128:(q + 1) * 128],
                        rhs_t[:, kt, :],
                        start=(i == 0),
                        stop=(i == 3),
                    )

            # D = C2^T = mm(cn, ct); C3 = C2@C = mm(ds, cn)
            pd = ps_d.tile([128, 2, 256], F32)
            mm4(pd, cn, ct)
            ds = sb_ds.tile([128, 2, 256], BF16, tag="ds")
            nc.scalar.copy(ds[:], pd[:])

            pc3 = ps_c3.tile([128, 2, 256], F32)
            mm4(pc3, ds, cn)
            c3 = sb_c3.tile([128, 512], BF16, tag="c3")
            nc.scalar.copy(c3[:], pc3[:])

            def dot(col, a, b):
                out = scr.tile([128, 2, 256], BF16, tag="scr")
                nc.vector.scalar_tensor_tensor(
                    out=out[:],
                    in0=a,
                    scalar=1.0,
                    in1=b,
                    op0=mybir.AluOpType.mult,
                    op1=mybir.AluOpType.mult,
                    accum_out=partials[:, col:col + 1],
                )

            dot(pair * 2 + 0, c3[:], ct)       # t4 = <C3, C^T>
            dot(pair * 2 + 1, c3[:], ds[:])    # t5 = <C3, C2^T>
            # t3 = <C2^T, C>: product on idle GpSimd, partition-reduce on PE
            # via a ones-column lhsT that routes this pair's sums to
            # partition `pair` of one persistent PSUM bank
            outg = scr.tile([128, 2, 256], BF16, tag="scrg", name="outg")
            nc.gpsimd.tensor_mul(outg[:], ds[:], cn)
            nc.tensor.matmul(
                pt3[:, :],
                oht[:, 63 - pair:127 - pair],
                outg[:].bitcast(BF16),
                start=(pair == 0),
                stop=(pair == npair - 1),
                skip_group_check=True,
            )

        nc.vector.tensor_copy(t3sb[:], pt3[:])
        nc.sync.dma_start(out=pa_d[:], in_=partials[:])
        nc.sync.dma_start(out=dg_d[:], in_=t3sb[:])

    nc.compile()
    _COMPILED = nc
    return nc


def kernel(x, conv_w, conv_b, coef):
    global LAST_EXEC_NS
    import ml_dtypes

    x = np.asarray(x, dtype=np.float32)
    conv_w = np.asarray(conv_w, dtype=np.float32)
    conv_b = np.asarray(conv_b, dtype=np.float32)
    coef = np.asarray(coef, dtype=np.float32)

    # --- host: conv via im2col GEMM ---
    from numpy.lib.stride_tricks import sliding_window_view
    win = sliding_window_view(x, (KK, KK), axis=(1, 2))      # [B,H,H,KK,KK]
    patches = np.ascontiguousarray(win).reshape(B, H * H, KK * KK)
    wmat = conv_w.reshape(CH, KK * KK)
    C = patches @ wmat.T                                      # [B, H*H, CH]
    C = C.transpose(0, 2, 1).reshape(B, CH, H, H) + conv_b[None, :, None, None]

    Cpad = np.zeros((B * CH, 256, 256), np.float32)
    Cpad[:, :H, :H] = C.reshape(B * CH, H, H)

    # t2 in full precision on host (the dominant-cancellation trace)
    t2 = np.einsum("pij,pji->p", Cpad.astype(np.float64), Cpad.astype(np.float64))

    # round once to bf16 (RNE), build both layouts from the same rounded values
    Cb = Cpad.astype(ml_dtypes.bfloat16)                      # [512,256,256]
    n = B * CH
    cn = Cb.reshape(n, 2, 128, 256).transpose(0, 2, 1, 3)
    ct = np.ascontiguousarray(Cb.transpose(0, 2, 1)).reshape(
        n, 2, 128, 256).transpose(0, 2, 1, 3)
    cc = np.ascontiguousarray(
        np.stack([cn, ct], axis=2))                           # [n,128,2,2,256]

    nc = _build()
    from concourse.bass_utils import run_bass_kernel_spmd

    npair = PAIRS_PER_CORE
    oh = np.zeros((128, 127), dtype=ml_dtypes.bfloat16)
    oh[:, 63] = 1
    in_maps = [{"cc": cc[c * npair:(c + 1) * npair], "oh": oh}
               for c in range(NCORES)]

    trace = os.environ.get("CONVTRACE_PROFILE", "0") == "1"
    if trace:
        import sys
        import types
        if "antenv.axon_hooks" not in sys.modules:
            import antenv  # noqa: F401
            from trn_agent_boot.trn_boot import _ntff_profile_via_ctypes
            hook = _ntff_profile_via_ctypes("/opt/axon/libaxon_pjrt.so")
            mod = types.ModuleType("antenv.axon_hooks")
            mod.get_axon_ntff_profile_hook = lambda: hook
            mod.set_axon_ntff_profile_hook = lambda h: None
            sys.modules["antenv.axon_hooks"] = mod
        import concourse.bass_utils as bu
        bu.upload_artifacts = lambda tmpdir: tmpdir

    res = run_bass_kernel_spmd(nc, in_maps, list(range(NCORES)), trace=trace)
    LAST_EXEC_NS = res.exec_time_ns

    # --- host: finalize in float64 ---
    ts = np.empty((B * CH, 4), np.float64)
    ts[:, 0] = t2
    for c in range(NCORES):
        pa = res.results[c]["pa"].astype(np.float64)           # [128, npair*2]
        t45 = pa.sum(axis=0).reshape(npair, 2)
        ts[c * npair:(c + 1) * npair, 2:] = t45
        dg = res.results[c]["dg"].astype(np.float64)           # [64, 512]
        ts[c * npair:(c + 1) * npair, 1] = dg.sum(axis=1)

    ts = ts.reshape(B, CH, 4)
    jpow = np.arange(1, COLS + 1, dtype=np.float64)
    retm = ts[..., None] ** jpow                               # [B,CH,ROWS,COLS]
    exps = (np.arange(ROWS, dtype=np.float64)[:, None]
            + np.arange(COLS, dtype=np.float64)[None, :] + 1.0)
    retm = retm / (np.float64(H * H) ** exps)
    out = (coef.astype(np.float64)[None] * retm).sum(axis=(1, 2, 3))
    return out.astype(np.float32)


# revision 9
# speedup vs baseline: 1.3062x; 1.1818x over previous
"""Trainium2 kernel for nn_ConvTrace: batch of 64 graphs, conv -> traces of
matrix powers -> coef-weighted sum.

Pipeline:
- Host: 6x6 conv via im2col GEMM (BLAS), zero-pad 251->256, round to bf16
  (RNE), pack natural+transposed layouts into one interleaved buffer, and
  compute t2 = tr(C^2) = <C, C^T> in full precision.
- Device (8 NeuronCores, data-parallel over the batch, 64 (b,ch) pairs/core):
  per pair, two bf16 matmul products on the PE (N=256 -> 1 cyc/row):
  D = C2^T = mm(lhsT=Cn, rhs=Ct) and C3 = C2@C = mm(lhsT=ds, rhs=Cn).
  ScalarE copies both PSUM results to SBUF bf16 (ds, c3). All three traces
  as DVE fused multiply-accumulate dots in bf16 2x mode:
  t3 = <ds, Cn>, t4 = <c3, Ct>, t5 = <c3, ds>. Per-partition partials DMA'd
  out once at the end.
- Host: reduce partials over partitions in float64 and apply the power/coef
  math.
"""

import os
from contextlib import ExitStack

import numpy as np

B = 64
G = 256
KK = 6
CH = 8
ROWS = 4
COLS = 3
H = G - KK + 1  # 251
NCORES = 8
PAIRS_PER_CORE = (B // NCORES) * CH  # 64

_COMPILED = None
LAST_EXEC_NS = None


def _build():
    """Build + compile the SPMD bass kernel once per process."""
    global _COMPILED
    if _COMPILED is not None:
        return _COMPILED

    import concourse.bacc as bacc
    import concourse.tile as tile
    from concourse import mybir

    F32 = mybir.dt.float32
    BF16 = mybir.dt.bfloat16
    npair = PAIRS_PER_CORE

    UI16 = mybir.dt.uint16

    nc = bacc.Bacc(None, target_bir_lowering=False)
    # interleaved natural (which=0) + transposed (which=1) layouts, one DMA/pair
    cc_d = nc.declare_dram_parameter("cc", [npair, 128, 2, 2, 256], BF16, isOutput=False)
    oh_d = nc.declare_dram_parameter("oh", [128, 127], BF16, isOutput=False)
    pa_d = nc.declare_dram_parameter("pa", [128, npair], F32, isOutput=True)
    dg_d = nc.declare_dram_parameter("dg", [64, 2, 512], F32, isOutput=True)

    with tile.TileContext(nc) as tc, ExitStack() as ctx:
        inp = ctx.enter_context(tc.tile_pool(name="inp", bufs=10))
        sb_ds = ctx.enter_context(tc.tile_pool(name="sb_ds", bufs=4))
        sb_c3 = ctx.enter_context(tc.tile_pool(name="sb_c3", bufs=4))
        scr = ctx.enter_context(tc.tile_pool(name="scr", bufs=6))
        pp = ctx.enter_context(tc.tile_pool(name="pp", bufs=1))
        ps_d = ctx.enter_context(tc.tile_pool(name="ps_d", bufs=3, space="PSUM"))
        ps_c3 = ctx.enter_context(tc.tile_pool(name="ps_c3", bufs=3, space="PSUM"))

        ps_t3 = ctx.enter_context(tc.tile_pool(name="ps_t3", bufs=1, space="PSUM"))
        ps_t4 = ctx.enter_context(tc.tile_pool(name="ps_t4", bufs=1, space="PSUM"))
        partials = pp.tile([128, npair], F32)
        t3sb = pp.tile([64, 2, 512], F32)
        oht = pp.tile([128, 127], BF16)
        nc.sync.dma_start(out=oht[:], in_=oh_d[:])
        pt3 = ps_t3.tile([64, 512], F32)
        pt4 = ps_t4.tile([64, 512], F32)

        for pair in range(npair):
            cct = inp.tile([128, 2, 2, 256], BF16, tag="cc")
            nc.sync.dma_start(out=cct[:], in_=cc_d[pair])
            cn = cct[:, 0]
            ct = cct[:, 1]

            def mm4(out_ps, lhs_t, rhs_t):
                # one PSUM accumulation group spanning the whole bank
                for i, (q, kt) in enumerate(((0, 0), (1, 0), (0, 1), (1, 1))):
                    nc.tensor.matmul(
                        out_ps[:, q, :],
                        lhs_t[:, kt, q * 128:(q + 1) * 128],
                        rhs_t[:, kt, :],
                        start=(i == 0),
                        stop=(i == 3),
                    )

            # D = C2^T = mm(cn, ct); C3 = C2@C = mm(ds, cn)
            pd = ps_d.tile([128, 2, 256], F32)
            mm4(pd, cn, ct)
            ds = sb_ds.tile([128, 2, 256], BF16, tag="ds")
            nc.scalar.copy(ds[:], pd[:])

            pc3 = ps_c3.tile([128, 2, 256], F32)
            mm4(pc3, ds, cn)
            c3 = sb_c3.tile([128, 2, 256], BF16, tag="c3")
            nc.scalar.copy(c3[:], pc3[:])

            def dot(col, a, b):
                out = scr.tile([128, 2, 256], BF16, tag="scr")
                nc.vector.scalar_tensor_tensor(
                    out=out[:],
                    in0=a,
                    scalar=1.0,
                    in1=b,
                    op0=mybir.AluOpType.mult,
                    op1=mybir.AluOpType.mult,
                    accum_out=partials[:, col:col + 1],
                )

            dot(pair, c3[:], ds[:])            # t5 = <C3, C2^T> (DVE stt)

            # t3 = <C2^T, C> and t4 = <C3, C^T>: products on DVE TT (2x bf16
            # mode), partition-reduce on PE via a ones-column lhsT that routes
            # this pair's sums to partition `pair` of a persistent PSUM bank
            def mul_reduce(bank, a, b):
                prod = scr.tile([128, 2, 256], BF16, tag="scrg", name="prod")
                nc.vector.tensor_mul(prod[:], a, b)
                nc.tensor.matmul(
                    bank[:, :],
                    oht[:, 63 - pair:127 - pair],
                    prod[:].bitcast(BF16),
                    start=(pair == 0),
                    stop=(pair == npair - 1),
                    skip_group_check=True,
                )

            mul_reduce(pt3, ds[:], cn)
            mul_reduce(pt4, c3[:], ct)

        nc.vector.tensor_copy(t3sb[:, 0, :], pt3[:])
        nc.vector.tensor_copy(t3sb[:, 1, :], pt4[:])
        nc.sync.dma_start(out=pa_d[:], in_=partials[:])
        nc.sync.dma_start(out=dg_d[:], in_=t3sb[:])

    nc.compile()
    _COMPILED = nc
    return nc


def kernel(x, conv_w, conv_b, coef):
    global LAST_EXEC_NS
    import ml_dtypes

    x = np.asarray(x, dtype=np.float32)
    conv_w = np.asarray(conv_w, dtype=np.float32)
    conv_b = np.asarray(conv_b, dtype=np.float32)
    coef = np.asarray(coef, dtype=np.float32)

    # --- host: conv via im2col GEMM ---
    from numpy.lib.stride_tricks import sliding_window_view
    win = sliding_window_view(x, (KK, KK), axis=(1, 2))      # [B,H,H,KK,KK]
    patches = np.ascontiguousarray(win).reshape(B, H * H, KK * KK)
    wmat = conv_w.reshape(CH, KK * KK)
    C = patches @ wmat.T                                      # [B, H*H, CH]
    C = C.transpose(0, 2, 1).reshape(B, CH, H, H) + conv_b[None, :, None, None]

    Cpad = np.zeros((B * CH, 256, 256), np.float32)
    Cpad[:, :H, :H] = C.reshape(B * CH, H, H)

    # t2 in full precision on host (the dominant-cancellation trace)
    t2 = np.einsum("pij,pji->p", Cpad.astype(np.float64), Cpad.astype(np.float64))

    # round once to bf16 (RNE), build both layouts from the same rounded values
    Cb = Cpad.astype(ml_dtypes.bfloat16)                      # [512,256,256]
    n = B * CH
    cn = Cb.reshape(n, 2, 128, 256).transpose(0, 2, 1, 3)
    ct = np.ascontiguousarray(Cb.transpose(0, 2, 1)).reshape(
        n, 2, 128, 256).transpose(0, 2, 1, 3)
    cc = np.ascontiguousarray(
        np.stack([cn, ct], axis=2))                           # [n,128,2,2,256]

    nc = _build()
    from concourse.bass_utils import run_bass_kernel_spmd

    npair = PAIRS_PER_CORE
    oh = np.zeros((128, 127), dtype=ml_dtypes.bfloat16)
    oh[:, 63] = 1
    in_maps = [{"cc": cc[c * npair:(c + 1) * npair], "oh": oh}
               for c in range(NCORES)]

    trace = os.environ.get("CONVTRACE_PROFILE", "0") == "1"
    if trace:
        import sys
        import types
        if "antenv.axon_hooks" not in sys.modules:
            import antenv  # noqa: F401
            from trn_agent_boot.trn_boot import _ntff_profile_via_ctypes
            hook = _ntff_profile_via_ctypes("/opt/axon/libaxon_pjrt.so")
            mod = types.ModuleType("antenv.axon_hooks")
            mod.get_axon_ntff_profile_hook = lambda: hook
            mod.set_axon_ntff_profile_hook = lambda h: None
            sys.modules["antenv.axon_hooks"] = mod
        import concourse.bass_utils as bu
        bu.upload_artifacts = lambda tmpdir: tmpdir

    res = run_bass_kernel_spmd(nc, in_maps, list(range(NCORES)), trace=trace)
    LAST_EXEC_NS = res.exec_time_ns

    # --- host: finalize in float64 ---
    ts = np.empty((B * CH, 4), np.float64)
    ts[:, 0] = t2
    for c in range(NCORES):
        pa = res.results[c]["pa"].astype(np.float64)           # [128, npair]
        ts[c * npair:(c + 1) * npair, 3] = pa.sum(axis=0)
        dg = res.results[c]["dg"].astype(np.float64)           # [64, 2, 512]
        ts[c * npair:(c + 1) * npair, 1] = dg[:, 0, :].sum(axis=1)
        ts[c * npair:(c + 1) * npair, 2] = dg[:, 1, :].sum(axis=1)

    ts = ts.reshape(B, CH, 4)
    jpow = np.arange(1, COLS + 1, dtype=np.float64)
    retm = ts[..., None] ** jpow                               # [B,CH,ROWS,COLS]
    exps = (np.arange(ROWS, dtype=np.float64)[:, None]
            + np.arange(COLS, dtype=np.float64)[None, :] + 1.0)
    retm = retm / (np.float64(H * H) ** exps)
    out = (coef.astype(np.float64)[None] * retm).sum(axis=(1, 2, 3))
    return out.astype(np.float32)
